# revision 35
# baseline (speedup 1.0000x reference)
import sys

sys.path.insert(0, "/opt/trn_rl_repo")
import numpy as np

import concourse.bacc as bacc
import concourse.tile as tile
from concourse import mybir
from concourse.bass_utils import run_bass_kernel_spmd

# nn_ColorShader: pytorch3d softmax_rgb_blend over K=10 faces/pixel,
# data-parallel over batch N=8 (one image per NeuronCore).
N, H, W, K = 8, 512, 512, 10
P = 128             # SBUF partitions
ROW = H * W // P    # 2048 pixels per partition row
T = 256             # pixels per tile chunk
NT = ROW // T       # tiles per core
SIGMA, GAMMA, EPS = 1e-4, 1e-4, 1e-10
ZNEAR, ZFAR = 1.0, 100.0
C = 2.0 ** -6       # gamma-units per zq quantization step
ZSCALE = 1.0 / ((ZFAR - ZNEAR) * GAMMA) / C  # zbuf units -> zq steps

f32 = mybir.dt.float32
f16 = mybir.dt.float16
bf16 = mybir.dt.bfloat16
u16 = mybir.dt.uint16
A = mybir.AluOpType
AF = mybir.ActivationFunctionType
AX = mybir.AxisListType

# Numerics vs reference.py (see simcheck.py):
# - host folds the pix>=0 mask into dx (=dists/SIGMA, invalid -> +80) and
#   zq (invalid -> 65535), so the pix tensor is never read on device.
# - zq = round((zbuf-ZNEAR)*ZSCALE) is z_inv/GAMMA quantized to 2^-6
#   gamma-steps, DECREASING in z_inv, so zqmin == z_inv_max and
#   weights_num = sigmoid(-dx)*exp(-C*(zq-zqmin)) elementwise.
# - any pixel with >=1 valid face has z_inv_max >= 0.9 so the reference
#   delta == 1e-10 exactly; all-invalid pixels also come out right because
#   every weight underflows and (S3+1e-10)/(den+1e-10) -> background = 1.
# - alpha = 1 - prod_k sigmoid(dx_k) via a pairwise fp16 product tree of
#   q = 1-p; invalid faces give q = 1 exactly.


def build():
    nc = bacc.Bacc("TRN2", target_bir_lowering=False, debug=False, num_devices=8)
    colors = nc.dram_tensor("colors", [P, ROW, 3, K], bf16, kind="ExternalInput").ap()
    dx = nc.dram_tensor("dx", [P, ROW, K], f16, kind="ExternalInput").ap()
    zq = nc.dram_tensor("zq", [P, ROW, K], u16, kind="ExternalInput").ap()
    out = nc.dram_tensor("out", [P, ROW, 4], f32, kind="ExternalOutput").ap()

    with tile.TileContext(nc) as tc:
        with tc.tile_pool(name="rows", bufs=1) as spool, \
             tc.tile_pool(name="work", bufs=2) as pool:
            prow = spool.tile([P, ROW, K], bf16)
            aprow = spool.tile([P, ROW], f32)
            bias_eps = spool.tile([P, 1], f32)
            nc.vector.memset(bias_eps, EPS)
            # Phase A: the sigmoid-table passes: p = sigmoid(-dx) for the
            # weights, q = sigmoid(dx) for alpha (scalar engine has slack).
            # First alpha-tree level on gpsimd, the small tail on DVE.
            for it in range(NT):
                s = slice(it * T, (it + 1) * T)
                dxt = pool.tile([P, T, K], f16)
                nc.sync.dma_start(out=dxt, in_=dx[:, s, :])
                nc.scalar.activation(prow[:, s, :], dxt, AF.Sigmoid, scale=-1.0)
                qt = pool.tile([P, T, K], f16)
                nc.scalar.activation(qt, dxt, AF.Sigmoid, scale=1.0)
                # alpha partial: prod_k q via pairwise tree (fp16)
                t1 = pool.tile([P, T, 5], f16)
                nc.gpsimd.tensor_tensor(t1, qt[:, :, 0:5], qt[:, :, 5:10], op=A.mult)
                t2 = pool.tile([P, T, 2], f16)
                nc.gpsimd.tensor_tensor(t2, t1[:, :, 0:2], t1[:, :, 2:4], op=A.mult)
                t3 = pool.tile([P, T, 1], f16)
                nc.gpsimd.tensor_tensor(t3, t2[:, :, 0:1], t2[:, :, 1:2], op=A.mult)
                nc.gpsimd.tensor_tensor(
                    aprow[:, s], t3[:, :, 0], t1[:, :, 4], op=A.mult
                )
            # Phase B: everything else lives in exp_and_others. All K-axis
            # reductions are pairwise tensor_tensor trees: TENSOR_REDUCE
            # never runs in the DVE 2x perf mode but 16-bit tensor_tensor
            # does, so trees are ~2x cheaper.
            for it in range(NT):
                s = slice(it * T, (it + 1) * T)
                zqt = pool.tile([P, T, K], u16)
                ctile = pool.tile([P, T, 3, K], bf16)
                nc.sync.dma_start(out=zqt, in_=zq[:, s, :])
                nc.sync.dma_start(out=ctile, in_=colors[:, s, :, :])
                # zqmin = min_k zq (exact u16 min)
                zqmin = pool.tile([P, T, 1], u16)
                nc.vector.tensor_reduce(zqmin[:, :, 0], zqt, axis=AX.X, op=A.min)
                # e2 = zq - zqmin >= 0; fp32 out keeps it exact (a rounded
                # e2 mis-scales weights that normalization can make
                # dominant -- costs ~1% output error, measured).
                e2 = pool.tile([P, T, K], u16)
                nc.vector.tensor_tensor(
                    e2, zqt, zqmin.broadcast_to([P, T, K]), op=A.subtract
                )
                EA = pool.tile([P, T, K], bf16)
                nc.scalar.activation(EA, e2, AF.Exp, scale=-C)
                E = pool.tile([P, T, 1, K], bf16)
                nc.vector.tensor_tensor(E[:, :, 0, :], EA, prow[:, s, :], op=A.mult)
                wc = pool.tile([P, T, 3, K], bf16, bufs=1)
                nc.vector.tensor_tensor(
                    wc, ctile, E.broadcast_to([P, T, 3, K]), op=A.mult
                )
                # S3 = sum_k wc via bf16 pairwise tree (2x mode; the tree
                # roundings are benign -- verified against the reference)
                r1 = pool.tile([P, T, 3, 5], bf16)
                nc.vector.tensor_tensor(
                    r1, wc[:, :, :, 0:5], wc[:, :, :, 5:10], op=A.add
                )
                r2 = pool.tile([P, T, 3, 2], bf16)
                nc.vector.tensor_tensor(
                    r2, r1[:, :, :, 0:2], r1[:, :, :, 2:4], op=A.add
                )
                r3 = pool.tile([P, T, 3, 1], bf16)
                nc.vector.tensor_tensor(
                    r3, r2[:, :, :, 0:1], r2[:, :, :, 1:2], op=A.add
                )
                S3 = pool.tile([P, T, 3], bf16)
                nc.vector.tensor_tensor(S3, r3[:, :, :, 0], r1[:, :, :, 4], op=A.add)
                # den = sum_k E (bf16 tree)
                d1 = pool.tile([P, T, 1, 5], bf16)
                nc.vector.tensor_tensor(
                    d1, E[:, :, :, 0:5], E[:, :, :, 5:10], op=A.add
                )
                d2 = pool.tile([P, T, 1, 2], bf16)
                nc.vector.tensor_tensor(
                    d2, d1[:, :, :, 0:2], d1[:, :, :, 2:4], op=A.add
                )
                d3 = pool.tile([P, T, 1, 1], bf16, tag="zqmin")
                nc.vector.tensor_tensor(
                    d3, d2[:, :, :, 0:1], d2[:, :, :, 1:2], op=A.add
                )
                den = pool.tile([P, T, 1], bf16, tag="zqmin")
                nc.vector.tensor_tensor(
                    den, d3[:, :, :, 0], d1[:, :, :, 4], op=A.add
                )
                sden = pool.tile([P, T], f32, tag="r2")
                nc.scalar.activation(sden, den[:, :, 0], AF.Identity, bias=bias_eps)
                rec = pool.tile([P, T, 1], f32, tag="d1")
                nc.vector.reciprocal_approx_fast(out=rec[:, :, 0], in_=sden)
                t1o = pool.tile([P, T, 3], f32, tag="r1")
                nc.scalar.activation(t1o, S3, AF.Identity, bias=bias_eps)
                otile = pool.tile([P, T, 4], f32, tag="ctile")
                nc.vector.tensor_tensor(
                    otile[:, :, 0:3], t1o, rec.broadcast_to([P, T, 3]), op=A.mult
                )
                nc.scalar.activation(
                    otile[:, :, 3], aprow[:, s], AF.Copy, scale=-1.0, bias=1.0
                )
                nc.scalar.dma_start(out=out[:, s, :], in_=otile)

    nc.compile()
    return nc


def make_in_maps(colors, pix_to_face, dists, zbuf):
    import ml_dtypes

    colors = np.asarray(colors)
    dists = np.asarray(dists, dtype=np.float32)
    zbuf = np.asarray(zbuf, dtype=np.float32)
    pix = np.asarray(pix_to_face)
    mask = pix >= 0
    dx = np.where(mask, dists * (1.0 / SIGMA), 80.0).astype(np.float16)
    zq = np.where(mask, np.rint((zbuf - ZNEAR) * ZSCALE), 65535.0).astype(np.uint16)
    in_maps = []
    for n in range(N):
        # [HW, K, 3] -> c-outer [P, ROW, 3, K] bf16 (uniform dtype with
        # the weights so the big multiply runs in the DVE 2x mode)
        ckt = np.ascontiguousarray(
            colors[n].reshape(P, ROW, K, 3).swapaxes(2, 3)
        ).astype(ml_dtypes.bfloat16)
        in_maps.append(
            {
                "colors": ckt,
                "dx": np.ascontiguousarray(dx[n].reshape(P, ROW, K)),
                "zq": np.ascontiguousarray(zq[n].reshape(P, ROW, K)),
            }
        )
    return in_maps


def assemble(results):
    outs = [results[n]["out"].reshape(H, W, 4) for n in range(N)]
    return np.stack(outs, axis=0).astype(np.float32)


_nc_cache = {}


def kernel(colors, pix_to_face, dists, zbuf):
    if "nc" not in _nc_cache:
        _nc_cache["nc"] = build()
    nc = _nc_cache["nc"]
    in_maps = make_in_maps(colors, pix_to_face, dists, zbuf)
    res = run_bass_kernel_spmd(nc, in_maps, list(range(N)))
    return assemble(res.results)
